# revision 14
# baseline (speedup 1.0000x reference)
"""Batch graph attention (GAT-style) Trainium2 kernel.

Problem: B=8, N=2048, F=64, FH=64, H=4.
  feats = X @ W[h]                         [B,H,N,FH]
  scores[n,m] = leaky_relu(s_self[n] + s_neigh[m], 0.2)
  P = softmax(scores + (1-A)*NEG_BIG, axis=m)
  out = relu(concat_h(P @ feats + b))

Sharding: batch b -> core b (8 cores, data parallel).

Per-core algorithm (transposed orientation: PE reduces over the neighbor
index m, which sits on SBUF partitions):

  exp(leaky(x)) == max(e^x, e^{0.2x})  (slope<1); dropping the per-column
  factor e^{s_self[n]} (softmax columns are scale invariant) leaves

      Phat[m,n] = A^T[m,n] * max(e1[m], e2[m] * g[n])

  with e1=0.5*exp(s_neigh), e2=0.5*exp(0.2*s_neigh), g=exp(-0.8*s_self)
  (the 0.5 keeps fp16 denominators comfortably in range; it cancels).
  Aggregation + denominators come from one PE matmul stream per m-tile:

      acc[o,n] += G[m,o]^T Phat[m,n],   G = [feats + b | 1]

  and out[n, h*64+o] = relu(acc[o,n] / acc[64,n]) is produced transposed
  ([H,FH,N] fp16 per core) and untransposed/cast on the host.

  A^T comes from fp32 A's fp16 bit-pair structure: fp16 view of fp32 1.0
  is [0x0000 | 0x3F80] = [0 | 1.875].  An xbar DMA transpose of 128 fp16
  columns starting at an ODD offset lands the 1.875*A values on EVEN
  output partitions (zeros on odd); the aligned window starting 128 later
  lands its values on ODD partitions.  One tensor_tensor add merges the
  two into a dense 128-partition tile holding 1.875*A^T with rows in the
  fixed interleave pi(p) = p/2 (p even) | 64+(p-1)/2 (p odd).  The 1.875
  cancels in the softmax; the pi permutation is absorbed by building
  XT16's columns pi-permuted (a permuted identity in the PE transposes),
  so G rows / e-vectors line up with no extra work.  g (an n-indexed row)
  is un-permuted during its PE transpose with the inverse identity.
"""

import numpy as np

B, N, F, FH, H = 8, 2048, 64, 64, 4
P = 128           # SBUF partitions
NT = N // P       # 16 m-tiles
C = 512           # matmul moving-operand chunk
NCH = N // C      # 4 chunks
GW = 66           # G row stride (64 feats + 1 ones + 1 pad)
LN_HALF = -0.6931471805599453

_CACHE = {}

# tuning knobs (read at build time)
KNOBS = {
    "tt_bufs": 6,         # xbar staging tile buffers
    "u_bufs": 4,
    "p_bufs": 6,
    "psu_bufs": 2,        # PSUM slot rotation (4 banks each)
    "outp_bufs": 3,
    "lead": 2,
    "merge_pool": 0,      # every k%merge_pool==merge_pool-1 merge on Pool (0=off)
    "merge_dma": 0,       # every k%merge_dma==merge_dma-1 merge via DMA (0=off)
    "p_pool_1": 4,        # phase-1: every n-th (h,k) p-op on Pool
    "p_pool_2": 3,        # phase-2: every n-th (h,k) p-op on Pool
    "gbc_dma": False,     # g broadcast via DMA (else Pool); DMA path rejects
                          # stride-0 partition APs, keep False
    "finals": "recip",    # "recip" (DVE approx recip) or "ln" (Act Ln/Exp)
    "out16": True,        # OUT dram tensor in fp16
}


def _build():
    if KNOBS.get("debug"):
        KNOBS.update({"tt_bufs": 3, "u_bufs": 2, "p_bufs": 3})
    import concourse.bacc as bacc
    import concourse.tile as tile
    import concourse.mybir as mybir
    from concourse.mybir import AluOpType as op, ActivationFunctionType as act

    f32 = mybir.dt.float32
    fp16 = mybir.dt.float16
    i32 = mybir.dt.int32

    nc = bacc.Bacc(
        "TRN2",
        target_bir_lowering=False,
        debug=False,
        enable_asserts=False,
        num_devices=8,
    )

    A_d = nc.dram_tensor("A", [N, N], f32, kind="ExternalInput").ap()
    X_d = nc.dram_tensor("X", [N, F], f32, kind="ExternalInput").ap()
    W_d = nc.dram_tensor("W", [H, F, FH], f32, kind="ExternalInput").ap()
    b_d = nc.dram_tensor("b", [H, FH], f32, kind="ExternalInput").ap()
    as_d = nc.dram_tensor("a_self", [H, FH], f32, kind="ExternalInput").ap()
    an_d = nc.dram_tensor("a_neigh", [H, FH], f32, kind="ExternalInput").ap()
    out_dt = fp16 if KNOBS["out16"] else f32
    OUT_d = nc.dram_tensor("OUT", [H, FH, N], out_dt, kind="ExternalOutput").ap()
    if KNOBS.get("debug"):
        DBG_den = nc.dram_tensor("DBG_den", [H, N], f32, kind="ExternalOutput").ap()
        DBG_rrow = nc.dram_tensor("DBG_rrow", [H, N], f32, kind="ExternalOutput").ap()
        DBG_AT = nc.dram_tensor("DBG_AT", [P, N], f32, kind="ExternalOutput").ap()
        DBG_U = nc.dram_tensor("DBG_U", [P, N], f32, kind="ExternalOutput").ap()
        DBG_E = nc.dram_tensor("DBG_E", [P, 3 * NT], f32, kind="ExternalOutput").ap()
        DBG_G = nc.dram_tensor("DBG_G", [P, NT * GW], f32, kind="ExternalOutput").ap()

    with tile.TileContext(nc) as tc:
        with (
            tc.tile_pool(name="const", bufs=1) as const,
            tc.tile_pool(name="big", bufs=1) as big,
            tc.tile_pool(name="stream", bufs=3) as stream,
            tc.tile_pool(name="head", bufs=2) as head,
            tc.tile_pool(name="outp", bufs=KNOBS["outp_bufs"]) as outp,
            tc.tile_pool(name="psu", bufs=KNOBS["psu_bufs"], space="PSUM") as psu,
        ):
            # ---- constants --------------------------------------------
            iota_i = const.tile([P, P], i32)
            nc.gpsimd.iota(iota_i[:], pattern=[[1, P]], base=0, channel_multiplier=0)
            pidx_i = const.tile([P, 1], i32)
            nc.gpsimd.iota(pidx_i[:], pattern=[[0, 1]], base=0, channel_multiplier=1)
            pidx_f = const.tile([P, 1], f32)
            nc.vector.tensor_copy(pidx_f[:], pidx_i[:])
            iota_f = const.tile([P, P], f32)
            nc.vector.tensor_copy(iota_f[:], iota_i[:])
            ident = const.tile([P, P], fp16)
            nc.vector.tensor_scalar(ident[:], iota_f[:], pidx_f[:], None, op.is_equal)
            # pi sequence [0,64,1,65,...]: ident_pi[p,c]=1 iff p==pi(c)
            iopi_i = const.tile([P, P], i32)
            nc.gpsimd.iota(iopi_i[:], pattern=[[1, 64], [64, 2]], base=0,
                           channel_multiplier=0)
            iopi_f = const.tile([P, P], f32)
            nc.vector.tensor_copy(iopi_f[:], iopi_i[:])
            ident_pi = const.tile([P, P], fp16)
            nc.vector.tensor_scalar(ident_pi[:], iopi_f[:], pidx_f[:], None,
                                    op.is_equal)
            # pinv sequence [0,2,..126,1,3,..127]: ident_pinv[p,c]=1 iff p==pinv(c)
            iopv_i = const.tile([P, P], i32)
            nc.gpsimd.iota(iopv_i[:], pattern=[[1, 2], [2, 64]], base=0,
                           channel_multiplier=0)
            iopv_f = const.tile([P, P], f32)
            nc.vector.tensor_copy(iopv_f[:], iopv_i[:])
            ident_pinv = const.tile([P, P], fp16)
            nc.vector.tensor_scalar(ident_pinv[:], iopv_f[:], pidx_f[:], None,
                                    op.is_equal)

            lnhalf = const.tile([P, 1], f32)
            nc.vector.memset(lnhalf[:], LN_HALF)

            # a_self / a_neigh as fp16 [64, H] columns (HWDGE + cast copy)
            avf = const.tile([F, 2 * H], f32)
            nc.sync.dma_start(avf[:, 0:H], as_d.rearrange("h o -> o h"))
            nc.sync.dma_start(avf[:, H : 2 * H], an_d.rearrange("h o -> o h"))
            av16 = const.tile([F, H], fp16)
            nc.vector.tensor_copy(av16[:], avf[:, 0:H])
            an16 = const.tile([F, H], fp16)
            nc.vector.tensor_copy(an16[:], avf[:, H : 2 * H])

            # ---- X -> XT16 [65, 2048] (fp16, pi-permuted cols, ones row 64)
            xf = const.tile([P, NT * F], f32)
            nc.sync.dma_start(
                xf.rearrange("p (t f) -> p t f", f=F),
                X_d.rearrange("(t p) f -> p t f", p=P),
            )
            x16 = const.tile([P, NT * F], fp16)
            nc.vector.tensor_copy(x16[:], xf[:])
            XT16 = big.tile([F + 1, N], fp16)
            xTps = psu.tile([F, N], fp16, tag="ps")
            for t in range(NT):
                nc.tensor.transpose(
                    xTps[:, t * P : (t + 1) * P],
                    x16[:, t * F : (t + 1) * F],
                    ident_pi[:],
                )
            nc.scalar.copy(XT16[0:F, :], xTps[:])
            nc.vector.memset(XT16[F : F + 1, :], 1.0)

            # ---- A^T via odd/even-offset fp16 xbar transposes + 1 merge ----
            AT_sb = big.tile([P, NT * N], fp16)
            Vf = A_d.bitcast(fp16)  # [2048, 4096]

            def emit_merge_tile(k):
                use_dma = KNOBS["merge_dma"] and k % KNOBS["merge_dma"] == (
                    KNOBS["merge_dma"] - 1)
                use_pool = (not use_dma) and KNOBS["merge_pool"] and (
                    k % KNOBS["merge_pool"] == KNOBS["merge_pool"] - 1)
                dst = AT_sb[:, k * N : (k + 1) * N]
                if use_dma:
                    # ta straight into AT_sb; tb's odd partitions DMA-merged in
                    nc.sync.dma_start_transpose(
                        dst, Vf[:, 256 * k + 1 : 256 * k + 129])
                    tb = stream.tile([P, N], fp16, tag="tt",
                                     bufs=KNOBS["tt_bufs"], name=f"tb_{k}")
                    nc.sync.dma_start_transpose(
                        tb[:], Vf[:, 256 * k + 128 : 256 * k + 256])
                    nc.sync.dma_start(
                        AT_sb[:][1:P:2, k * N : (k + 1) * N], tb[:][1:P:2, :])
                else:
                    ta = stream.tile([P, N], fp16, tag="tt",
                                     bufs=KNOBS["tt_bufs"], name=f"ta_{k}")
                    nc.sync.dma_start_transpose(
                        ta[:], Vf[:, 256 * k + 1 : 256 * k + 129])
                    tb = stream.tile([P, N], fp16, tag="tt",
                                     bufs=KNOBS["tt_bufs"], name=f"tb_{k}")
                    nc.sync.dma_start_transpose(
                        tb[:], Vf[:, 256 * k + 128 : 256 * k + 256])
                    eng = nc.gpsimd if use_pool else nc.vector
                    eng.tensor_tensor(dst, ta[:], tb[:], op.add)

            def emit_setup(h):
                # [W[h]; b[h]] as fp16 [65, 64] (SWDGE cast DMA)
                W16 = head.tile([F + 1, FH], fp16, tag="W16", bufs=2,
                                name=f"W16_{h}")
                nc.gpsimd.dma_start(W16[0:F, :], W_d[h])
                nc.gpsimd.dma_start(W16[F : F + 1, :], b_d[h : h + 1, :])

                featsT = head.tile([FH, N], fp16, tag="featsT", bufs=2,
                                   name=f"featsT_{h}")
                for c in range(NCH):
                    sl = slice(c * C, (c + 1) * C)
                    psF = psu.tile([FH, C], f32, tag="ps", name=f"psF_{h}_{c}")
                    nc.tensor.matmul(
                        psF[:], W16[0:F, :], XT16[0:F, sl],
                        start=True, stop=True,
                    )
                    nc.scalar.copy(featsT[:, sl], psF[:])

                psNg = psu.tile([P, 2 * NT], f32, tag="ps", name=f"psNg_{h}")
                for k in range(NT):
                    nc.tensor.matmul(
                        psNg[:, k : k + 1],
                        featsT[:, k * P : (k + 1) * P],
                        an16[:, h : h + 1],
                        start=True, stop=True,
                    )
                    nc.tensor.matmul(
                        psNg[:, NT + k : NT + k + 1],
                        featsT[:, k * P : (k + 1) * P],
                        av16[:, h : h + 1],
                        start=True, stop=True,
                    )
                # e1 = 0.5*exp(s_neigh), e2 = 0.5*exp(0.2*s_neigh)  (pi rows)
                e1g = head.tile([P, NT], f32, tag="e1g", bufs=2, name=f"e1g_{h}")
                nc.scalar.activation(e1g[:], psNg[:, 0:NT], act.Exp,
                                     scale=1.0, bias=lnhalf[:])
                e2g = head.tile([P, NT], f32, tag="e2g", bufs=2, name=f"e2g_{h}")
                nc.scalar.activation(e2g[:], psNg[:, 0:NT], act.Exp,
                                     scale=0.2, bias=lnhalf[:])
                ssg = head.tile([P, NT], fp16, tag="ssg", bufs=2, name=f"ssg_{h}")
                nc.scalar.copy(ssg[:], psNg[:, NT : 2 * NT])

                # g_row natural order: un-permute ssg with ident_pinv
                g_row = head.tile([1, N], fp16, tag="g_row", bufs=2,
                                  name=f"g_row_{h}")
                for c in range(NCH):
                    psRow = psu.tile([1, C], fp16, tag="ps", name=f"psRow_{h}_{c}")
                    for j in range(4):
                        kk = c * 4 + j
                        nc.tensor.transpose(
                            psRow[:, j * P : (j + 1) * P],
                            ssg[:, kk : kk + 1],
                            ident_pinv[:],
                        )
                    nc.scalar.activation(
                        g_row[:, c * C : (c + 1) * C], psRow[:], act.Exp,
                        scale=-0.8,
                    )
                g_bc = head.tile([P, N], fp16, tag="g_bc", bufs=2, name=f"g_bc_{h}")
                if KNOBS["gbc_dma"]:
                    nc.sync.dma_start(
                        g_bc[:], g_row[:].partition_broadcast(P).squeeze(1))
                else:
                    nc.gpsimd.partition_broadcast(g_bc[:], g_row[:])

                G_all = head.tile([P, NT * GW], fp16, tag="G_all", bufs=2,
                                  name=f"G_all_{h}")
                G3 = G_all.rearrange("p (k w) -> p k w", w=GW)
                for halfg in range(2):
                    psG = psu.tile([P, (NT // 2) * FH], f32, tag="ps",
                                   name=f"psG_{h}_{halfg}")
                    for j in range(NT // 2):
                        k = halfg * (NT // 2) + j
                        nc.tensor.matmul(
                            psG[:, j * FH : (j + 1) * FH],
                            XT16[:, k * P : (k + 1) * P],
                            W16[:],
                            start=True, stop=True,
                        )
                    nc.scalar.copy(
                        G3[:, halfg * (NT // 2) : (halfg + 1) * (NT // 2), 0:FH],
                        psG.rearrange("p (k f) -> p k f", f=FH),
                    )
                nc.vector.memset(G3[:, :, FH : FH + 1], 1.0)
                agg = psu.tile([FH + 1, N], f32, tag="ps", name=f"agg{h}")
                return (e1g, e2g, g_bc, G_all, agg)

            pcount = [0]

            def emit_up(h, st, k, pool_stride):
                e1g, e2g, g_bc, G_all, agg = st
                u_t = stream.tile([P, N], fp16, tag="u", bufs=KNOBS["u_bufs"],
                                  name=f"u_{h}_{k}")
                nc.vector.tensor_scalar(
                    u_t[:], g_bc[:],
                    e2g[:, k : k + 1], e1g[:, k : k + 1],
                    op.mult, op.max,
                )
                if KNOBS.get("debug") and h == 0 and k == 0:
                    dbg = big.tile([P, N], f32, name="dbgu")
                    nc.vector.tensor_copy(dbg[:], u_t[:])
                    nc.sync.dma_start(DBG_U[:, :], dbg[:])
                    dbg2 = big.tile([P, N], f32, name="dbgat")
                    nc.vector.tensor_copy(dbg2[:], AT_sb[:, 0:N])
                    nc.sync.dma_start(DBG_AT[:, :], dbg2[:])
                    dbg3 = big.tile([P, 3 * NT], f32, name="dbge")
                    nc.vector.tensor_copy(dbg3[:, 0:NT], e1g[:])
                    nc.vector.tensor_copy(dbg3[:, NT : 2 * NT], e2g[:])
                    nc.vector.tensor_copy(dbg3[:, 2 * NT : 3 * NT], g_bc[:, 0:NT])
                    nc.sync.dma_start(DBG_E[:, :], dbg3[:])
                    dbg4 = big.tile([P, NT * GW], f32, name="dbgg")
                    nc.vector.tensor_copy(dbg4[:], G_all[:])
                    nc.sync.dma_start(DBG_G[:, :], dbg4[:])
                p_t = stream.tile([P, N], fp16, tag="p", bufs=KNOBS["p_bufs"],
                                  name=f"p_{h}_{k}")
                pcount[0] += 1
                eng = (nc.gpsimd if (pool_stride
                       and pcount[0] % pool_stride == pool_stride - 1)
                       else nc.vector)
                eng.tensor_tensor(
                    p_t[:], u_t[:], AT_sb[:, k * N : (k + 1) * N], op.mult
                )
                return p_t

            def emit_aggs(h, st, k, p_t):
                e1g, e2g, g_bc, G_all, agg = st
                for c in range(NCH):
                    sl = slice(c * C, (c + 1) * C)
                    nc.tensor.matmul(
                        agg[:, sl],
                        G_all[:, k * GW : k * GW + FH + 1],
                        p_t[:, sl],
                        start=(k == 0), stop=(k == NT - 1),
                    )

            def emit_main_tile(h, st, k, pool_stride):
                emit_aggs(h, st, k, emit_up(h, st, k, pool_stride))

            def emit_finals(h, st):
                e1g, e2g, g_bc, G_all, agg = st
                rrow = head.tile([1, N], f32, tag="rrow", bufs=1,
                                 name=f"rrow_{h}")
                if KNOBS["finals"] == "recip":
                    # custom-DVE recip misreads PSUM rows at partition
                    # offset 64 -> bounce den through SBUF partition 0
                    den_sb = head.tile([1, N], f32, tag="den_sb", bufs=1,
                                       name=f"den_sb_{h}")
                    nc.scalar.copy(den_sb[:], agg[FH : FH + 1, :])
                    if KNOBS.get("debug"):
                        nc.sync.dma_start(DBG_den[h : h + 1, :], den_sb[:])
                    nc.vector.reciprocal_approx_fast(rrow[:], den_sb[:])
                else:
                    lnr = head.tile([1, N], f32, tag="lnr", bufs=2,
                                    name=f"lnr_{h}")
                    nc.scalar.activation(lnr[:], agg[FH : FH + 1, :], act.Ln)
                    nc.scalar.activation(rrow[:], lnr[:], act.Exp, scale=-1.0)
                if KNOBS.get("debug"):
                    nc.sync.dma_start(DBG_rrow[h : h + 1, :], rrow[:])
                rbc = head.tile([FH, N], f32, tag="rbc", bufs=1,
                                name=f"rbc_{h}")
                nc.gpsimd.partition_broadcast(rbc[:], rrow[:])
                for c in range(NCH):
                    sl = slice(c * C, (c + 1) * C)
                    outf = outp.tile([FH, C], out_dt, tag="outf",
                                     name=f"outf_{h}_{c}")
                    nc.vector.scalar_tensor_tensor(
                        outf[:],
                        agg[0:FH, sl],
                        0.0, rbc[:, sl], op.max, op.mult,
                    )
                    nc.sync.dma_start(OUT_d[h, :, sl], outf[:])

            # ---- schedule: heads 0/1 ride the A^T build; 2/3 follow ----
            sts = [None] * H
            sts[0] = emit_setup(0)
            sts[1] = emit_setup(1)
            lead = KNOBS["lead"]
            for k in range(NT):
                emit_merge_tile(k)
                if k >= lead:
                    emit_main_tile(0, sts[0], k - lead, KNOBS["p_pool_1"])
                    emit_main_tile(1, sts[1], k - lead, KNOBS["p_pool_1"])
            for k in range(NT - lead, NT):
                emit_main_tile(0, sts[0], k, KNOBS["p_pool_1"])
                emit_main_tile(1, sts[1], k, KNOBS["p_pool_1"])
            emit_finals(0, sts[0])
            sts[2] = emit_setup(2)
            emit_finals(1, sts[1])
            sts[3] = emit_setup(3)
            for k in range(NT):
                emit_main_tile(2, sts[2], k, KNOBS["p_pool_2"])
                emit_main_tile(3, sts[3], k, KNOBS["p_pool_2"])
            emit_finals(2, sts[2])
            emit_finals(3, sts[3])

    nc.compile()
    return nc


def _get_nc():
    if "nc" not in _CACHE:
        _CACHE["nc"] = _build()
    return _CACHE["nc"]


def make_in_maps(inputs):
    X = np.ascontiguousarray(inputs["X"], dtype=np.float32)
    A = np.ascontiguousarray(inputs["A"], dtype=np.float32)
    W = np.ascontiguousarray(inputs["W"], dtype=np.float32)
    b = np.ascontiguousarray(inputs["b"], dtype=np.float32)
    a_self = np.ascontiguousarray(inputs["a_self"], dtype=np.float32)
    a_neigh = np.ascontiguousarray(inputs["a_neigh"], dtype=np.float32)
    return [
        {
            "A": np.ascontiguousarray(A[i]),
            "X": np.ascontiguousarray(X[i]),
            "W": W,
            "b": b,
            "a_self": a_self,
            "a_neigh": a_neigh,
        }
        for i in range(B)
    ]


def run(inputs, trace=False):
    from concourse import bass_utils

    nc = _get_nc()
    in_maps = make_in_maps(inputs)
    res = bass_utils.run_bass_kernel_spmd(
        nc, in_maps, core_ids=list(range(B)), trace=trace
    )
    out = np.empty((B, N, H * FH), dtype=np.float32)
    for i in range(B):
        o = np.asarray(res.results[i]["OUT"], dtype=np.float32)  # [H, FH, N]
        out[i] = o.transpose(2, 0, 1).reshape(N, H * FH)
    return out, res


def kernel(**inputs):
    out, _ = run(inputs, trace=False)
    return out


# revision 18
# speedup vs baseline: 1.0740x; 1.0740x over previous
"""Batch graph attention (GAT-style) Trainium2 kernel.

Problem: B=8, N=2048, F=64, FH=64, H=4.
  feats = X @ W[h]                         [B,H,N,FH]
  scores[n,m] = leaky_relu(s_self[n] + s_neigh[m], 0.2)
  P = softmax(scores + (1-A)*NEG_BIG, axis=m)
  out = relu(concat_h(P @ feats + b))

Sharding: batch b -> core b (8 cores, data parallel).

Per-core algorithm (transposed orientation: PE reduces over the neighbor
index m, which sits on SBUF partitions):

  exp(leaky(x)) == max(e^x, e^{0.2x})  (slope<1); dropping the per-column
  factor e^{s_self[n]} (softmax columns are scale invariant) leaves

      Phat[m,n] = A^T[m,n] * max(e1[m], e2[m] * g[n])

  with e1=0.5*exp(s_neigh), e2=0.5*exp(0.2*s_neigh), g=exp(-0.8*s_self)
  (the 0.5 keeps fp16 denominators comfortably in range; it cancels).
  Aggregation + denominators come from one PE matmul stream per m-tile:

      acc[o,n] += G[m,o]^T Phat[m,n],   G = [feats + b | 1]

  and out[n, h*64+o] = relu(acc[o,n] / acc[64,n]) is produced transposed
  ([H,FH,N] fp16 per core) and untransposed/cast on the host.

  A^T comes from fp32 A's fp16 bit-pair structure: fp16 view of fp32 1.0
  is [0x0000 | 0x3F80] = [0 | 1.875].  An xbar DMA transpose of 128 fp16
  columns starting at an ODD offset lands the 1.875*A values on EVEN
  output partitions (zeros on odd); the aligned window starting 128 later
  lands its values on ODD partitions.  One tensor_tensor add merges the
  two into a dense 128-partition tile holding 1.875*A^T with rows in the
  fixed interleave pi(p) = p/2 (p even) | 64+(p-1)/2 (p odd).  The 1.875
  cancels in the softmax; the pi permutation is absorbed by building
  XT16's columns pi-permuted (a permuted identity in the PE transposes),
  so G rows / e-vectors line up with no extra work.  g (an n-indexed row)
  is un-permuted during its PE transpose with the inverse identity.
"""

import numpy as np

B, N, F, FH, H = 8, 2048, 64, 64, 4
P = 128           # SBUF partitions
NT = N // P       # 16 m-tiles
C = 512           # matmul moving-operand chunk
NCH = N // C      # 4 chunks
GW = 66           # G row stride (64 feats + 1 ones + 1 pad)
LN_HALF = -0.6931471805599453

_CACHE = {}

# tuning knobs (read at build time)
KNOBS = {
    "tt_bufs": 6,         # xbar staging tile buffers
    "u_bufs": 3,
    "p_bufs": 6,
    "psu_bufs": 2,        # PSUM slot rotation (4 banks each)
    "outp_bufs": 3,
    "lead": 2,
    "pool_la": 4,     # lookahead (in seq steps) for pool-assigned p ops
    "merge_pool": 0,      # every k%merge_pool==merge_pool-1 merge on Pool (0=off)
    "merge_dma": 0,       # every k%merge_dma==merge_dma-1 merge via DMA (0=off)
    "p_pool_1": 4,        # phase-1: every n-th (h,k) p-op on Pool
    "p_pool_2": 4,        # phase-2: every n-th (h,k) p-op on Pool
    "gbc_dma": False,     # g broadcast via DMA (else Pool); DMA path rejects
                          # stride-0 partition APs, keep False
    "finals": "recip",    # "recip" (DVE approx recip) or "ln" (Act Ln/Exp)
    "out16": True,        # OUT dram tensor in fp16
}


def _build():
    if KNOBS.get("debug"):
        KNOBS.update({"tt_bufs": 3, "u_bufs": 2, "p_bufs": 3})
    import concourse.bacc as bacc
    import concourse.tile as tile
    import concourse.mybir as mybir
    from concourse.mybir import AluOpType as op, ActivationFunctionType as act

    f32 = mybir.dt.float32
    fp16 = mybir.dt.float16
    i32 = mybir.dt.int32

    nc = bacc.Bacc(
        "TRN2",
        target_bir_lowering=False,
        debug=False,
        enable_asserts=False,
        num_devices=8,
    )

    A_d = nc.dram_tensor("A", [N, N], f32, kind="ExternalInput").ap()
    X_d = nc.dram_tensor("X", [N, F], f32, kind="ExternalInput").ap()
    W_d = nc.dram_tensor("W", [H, F, FH], f32, kind="ExternalInput").ap()
    b_d = nc.dram_tensor("b", [H, FH], f32, kind="ExternalInput").ap()
    as_d = nc.dram_tensor("a_self", [H, FH], f32, kind="ExternalInput").ap()
    an_d = nc.dram_tensor("a_neigh", [H, FH], f32, kind="ExternalInput").ap()
    out_dt = fp16 if KNOBS["out16"] else f32
    OUT_d = nc.dram_tensor("OUT", [H, FH, N], out_dt, kind="ExternalOutput").ap()
    if KNOBS.get("debug"):
        DBG_den = nc.dram_tensor("DBG_den", [H, N], f32, kind="ExternalOutput").ap()
        DBG_rrow = nc.dram_tensor("DBG_rrow", [H, N], f32, kind="ExternalOutput").ap()
        DBG_AT = nc.dram_tensor("DBG_AT", [P, N], f32, kind="ExternalOutput").ap()
        DBG_U = nc.dram_tensor("DBG_U", [P, N], f32, kind="ExternalOutput").ap()
        DBG_E = nc.dram_tensor("DBG_E", [P, 3 * NT], f32, kind="ExternalOutput").ap()
        DBG_G = nc.dram_tensor("DBG_G", [P, NT * GW], f32, kind="ExternalOutput").ap()

    with tile.TileContext(nc) as tc:
        with (
            tc.tile_pool(name="const", bufs=1) as const,
            tc.tile_pool(name="big", bufs=1) as big,
            tc.tile_pool(name="stream", bufs=3) as stream,
            tc.tile_pool(name="head", bufs=2) as head,
            tc.tile_pool(name="outp", bufs=KNOBS["outp_bufs"]) as outp,
            tc.tile_pool(name="psu", bufs=KNOBS["psu_bufs"], space="PSUM") as psu,
        ):
            # ---- constants --------------------------------------------
            iota_i = stream.tile([P, P], i32, tag="cst", bufs=2)
            nc.gpsimd.iota(iota_i[:], pattern=[[1, P]], base=0, channel_multiplier=0)
            pidx_i = stream.tile([P, 1], i32, tag="cst1", bufs=1)
            nc.gpsimd.iota(pidx_i[:], pattern=[[0, 1]], base=0, channel_multiplier=1)
            pidx_f = const.tile([P, 1], f32)
            nc.vector.tensor_copy(pidx_f[:], pidx_i[:])
            iota_f = stream.tile([P, P], f32, tag="cstf", bufs=2)
            nc.vector.tensor_copy(iota_f[:], iota_i[:])
            ident = const.tile([P, P], fp16)
            nc.vector.tensor_scalar(ident[:], iota_f[:], pidx_f[:], None, op.is_equal)
            # pi sequence [0,64,1,65,...]: ident_pi[p,c]=1 iff p==pi(c)
            iopi_i = stream.tile([P, P], i32, tag="cst", bufs=2)
            nc.gpsimd.iota(iopi_i[:], pattern=[[1, 64], [64, 2]], base=0,
                           channel_multiplier=0)
            iopi_f = stream.tile([P, P], f32, tag="cstf", bufs=2)
            nc.vector.tensor_copy(iopi_f[:], iopi_i[:])
            ident_pi = const.tile([P, P], fp16)
            nc.vector.tensor_scalar(ident_pi[:], iopi_f[:], pidx_f[:], None,
                                    op.is_equal)
            # pinv sequence [0,2,..126,1,3,..127]: ident_pinv[p,c]=1 iff p==pinv(c)
            iopv_i = stream.tile([P, P], i32, tag="cst", bufs=2)
            nc.gpsimd.iota(iopv_i[:], pattern=[[1, 2], [2, 64]], base=0,
                           channel_multiplier=0)
            iopv_f = stream.tile([P, P], f32, tag="cstf", bufs=2)
            nc.vector.tensor_copy(iopv_f[:], iopv_i[:])
            ident_pinv = const.tile([P, P], fp16)
            nc.vector.tensor_scalar(ident_pinv[:], iopv_f[:], pidx_f[:], None,
                                    op.is_equal)

            lnhalf = const.tile([P, 1], f32)
            nc.vector.memset(lnhalf[:], LN_HALF)

            # a_self / a_neigh as fp16 [64, H] columns (HWDGE + cast copy)
            avf = stream.tile([F, 2 * H], f32, tag="avf", bufs=1)
            nc.sync.dma_start(avf[:, 0:H], as_d.rearrange("h o -> o h"))
            nc.sync.dma_start(avf[:, H : 2 * H], an_d.rearrange("h o -> o h"))
            av16 = const.tile([F, H], fp16)
            nc.vector.tensor_copy(av16[:], avf[:, 0:H])
            an16 = const.tile([F, H], fp16)
            nc.vector.tensor_copy(an16[:], avf[:, H : 2 * H])

            # ---- X -> XT16 [65, 2048] (fp16, pi-permuted cols, ones row 64)
            xf = stream.tile([P, NT * F], f32, tag="xf", bufs=1)
            nc.sync.dma_start(
                xf.rearrange("p (t f) -> p t f", f=F),
                X_d.rearrange("(t p) f -> p t f", p=P),
            )
            x16 = stream.tile([P, NT * F], fp16, tag="x16", bufs=1)
            nc.vector.tensor_copy(x16[:], xf[:])
            XT16 = big.tile([F + 1, N], fp16)
            xTps = psu.tile([F, N], fp16, tag="ps")
            for t in range(NT):
                nc.tensor.transpose(
                    xTps[:, t * P : (t + 1) * P],
                    x16[:, t * F : (t + 1) * F],
                    ident_pi[:],
                )
            nc.scalar.copy(XT16[0:F, :], xTps[:])
            nc.vector.memset(XT16[F : F + 1, :], 1.0)

            # ---- A^T via odd/even-offset fp16 xbar transposes + 1 merge ----
            AT_sb = big.tile([P, NT * N], fp16)
            Vf = A_d.bitcast(fp16)  # [2048, 4096]

            def emit_merge_tile(k):
                use_dma = KNOBS["merge_dma"] and k % KNOBS["merge_dma"] == (
                    KNOBS["merge_dma"] - 1)
                use_pool = (not use_dma) and KNOBS["merge_pool"] and (
                    k % KNOBS["merge_pool"] == KNOBS["merge_pool"] - 1)
                dst = AT_sb[:, k * N : (k + 1) * N]
                if use_dma:
                    # ta straight into AT_sb; tb's odd partitions DMA-merged in
                    nc.sync.dma_start_transpose(
                        dst, Vf[:, 256 * k + 1 : 256 * k + 129])
                    tb = stream.tile([P, N], fp16, tag="tt",
                                     bufs=KNOBS["tt_bufs"], name=f"tb_{k}")
                    nc.sync.dma_start_transpose(
                        tb[:], Vf[:, 256 * k + 128 : 256 * k + 256])
                    nc.sync.dma_start(
                        AT_sb[:][1:P:2, k * N : (k + 1) * N], tb[:][1:P:2, :])
                else:
                    ta = stream.tile([P, N], fp16, tag="tt",
                                     bufs=KNOBS["tt_bufs"], name=f"ta_{k}")
                    nc.sync.dma_start_transpose(
                        ta[:], Vf[:, 256 * k + 1 : 256 * k + 129])
                    tb = stream.tile([P, N], fp16, tag="tt",
                                     bufs=KNOBS["tt_bufs"], name=f"tb_{k}")
                    nc.sync.dma_start_transpose(
                        tb[:], Vf[:, 256 * k + 128 : 256 * k + 256])
                    eng = nc.gpsimd if use_pool else nc.vector
                    eng.tensor_tensor(dst, ta[:], tb[:], op.add)

            def emit_setup(h):
                # [W[h]; b[h]] as fp16 [65, 64] (SWDGE cast DMA)
                W16 = head.tile([F + 1, FH], fp16, tag="W16", bufs=2,
                                name=f"W16_{h}")
                nc.gpsimd.dma_start(W16[0:F, :], W_d[h])
                nc.gpsimd.dma_start(W16[F : F + 1, :], b_d[h : h + 1, :])

                featsT = head.tile([FH, N], fp16, tag="featsT", bufs=2,
                                   name=f"featsT_{h}")
                for c in range(NCH):
                    sl = slice(c * C, (c + 1) * C)
                    psF = psu.tile([FH, C], f32, tag="ps", name=f"psF_{h}_{c}")
                    nc.tensor.matmul(
                        psF[:], W16[0:F, :], XT16[0:F, sl],
                        start=True, stop=True,
                    )
                    nc.scalar.copy(featsT[:, sl], psF[:])

                psNg = psu.tile([P, 2 * NT], f32, tag="ps", name=f"psNg_{h}")
                for k in range(NT):
                    nc.tensor.matmul(
                        psNg[:, k : k + 1],
                        featsT[:, k * P : (k + 1) * P],
                        an16[:, h : h + 1],
                        start=True, stop=True,
                    )
                    nc.tensor.matmul(
                        psNg[:, NT + k : NT + k + 1],
                        featsT[:, k * P : (k + 1) * P],
                        av16[:, h : h + 1],
                        start=True, stop=True,
                    )
                # e1 = 0.5*exp(s_neigh), e2 = 0.5*exp(0.2*s_neigh)  (pi rows)
                e1g = head.tile([P, NT], f32, tag="e1g", bufs=4, name=f"e1g_{h}")
                nc.scalar.activation(e1g[:], psNg[:, 0:NT], act.Exp,
                                     scale=1.0, bias=lnhalf[:])
                e2g = head.tile([P, NT], f32, tag="e2g", bufs=4, name=f"e2g_{h}")
                nc.scalar.activation(e2g[:], psNg[:, 0:NT], act.Exp,
                                     scale=0.2, bias=lnhalf[:])
                ssg = head.tile([P, NT], fp16, tag="ssg", bufs=2, name=f"ssg_{h}")
                nc.scalar.copy(ssg[:], psNg[:, NT : 2 * NT])

                # g_row natural order: un-permute ssg with ident_pinv
                g_row = head.tile([1, N], fp16, tag="g_row", bufs=2,
                                  name=f"g_row_{h}")
                for c in range(NCH):
                    psRow = psu.tile([1, C], fp16, tag="ps", name=f"psRow_{h}_{c}")
                    for j in range(4):
                        kk = c * 4 + j
                        nc.tensor.transpose(
                            psRow[:, j * P : (j + 1) * P],
                            ssg[:, kk : kk + 1],
                            ident_pinv[:],
                        )
                    nc.scalar.activation(
                        g_row[:, c * C : (c + 1) * C], psRow[:], act.Exp,
                        scale=-0.8,
                    )
                g_bc = head.tile([P, N], fp16, tag="g_bc", bufs=4, name=f"g_bc_{h}")
                if KNOBS["gbc_dma"]:
                    nc.sync.dma_start(
                        g_bc[:], g_row[:].partition_broadcast(P).squeeze(1))
                else:
                    nc.gpsimd.partition_broadcast(g_bc[:], g_row[:])

                G_all = head.tile([P, NT * GW], fp16, tag="G_all", bufs=4,
                                  name=f"G_all_{h}")
                G3 = G_all.rearrange("p (k w) -> p k w", w=GW)
                for halfg in range(2):
                    psG = psu.tile([P, (NT // 2) * FH], f32, tag="ps",
                                   name=f"psG_{h}_{halfg}")
                    for j in range(NT // 2):
                        k = halfg * (NT // 2) + j
                        nc.tensor.matmul(
                            psG[:, j * FH : (j + 1) * FH],
                            XT16[:, k * P : (k + 1) * P],
                            W16[:],
                            start=True, stop=True,
                        )
                    nc.scalar.copy(
                        G3[:, halfg * (NT // 2) : (halfg + 1) * (NT // 2), 0:FH],
                        psG.rearrange("p (k f) -> p k f", f=FH),
                    )
                nc.vector.memset(G3[:, :, FH : FH + 1], 1.0)
                return {"e1g": e1g, "e2g": e2g, "g_bc": g_bc,
                        "G_all": G_all, "agg": None}

            pcount = [0]

            def emit_up(h, st, k, use_pool):
                e1g, e2g, g_bc = st["e1g"], st["e2g"], st["g_bc"]
                u_t = stream.tile([P, N], fp16, tag="u", bufs=KNOBS["u_bufs"],
                                  name=f"u_{h}_{k}")
                nc.vector.tensor_scalar(
                    u_t[:], g_bc[:],
                    e2g[:, k : k + 1], e1g[:, k : k + 1],
                    op.mult, op.max,
                )
                if KNOBS.get("debug") and h == 0 and k == 0:
                    dbg = big.tile([P, N], f32, name="dbgu")
                    nc.vector.tensor_copy(dbg[:], u_t[:])
                    nc.sync.dma_start(DBG_U[:, :], dbg[:])
                    dbg2 = big.tile([P, N], f32, name="dbgat")
                    nc.vector.tensor_copy(dbg2[:], AT_sb[:, 0:N])
                    nc.sync.dma_start(DBG_AT[:, :], dbg2[:])
                    dbg3 = big.tile([P, 3 * NT], f32, name="dbge")
                    nc.vector.tensor_copy(dbg3[:, 0:NT], e1g[:])
                    nc.vector.tensor_copy(dbg3[:, NT : 2 * NT], e2g[:])
                    nc.vector.tensor_copy(dbg3[:, 2 * NT : 3 * NT], g_bc[:, 0:NT])
                    nc.sync.dma_start(DBG_E[:, :], dbg3[:])
                    dbg4 = big.tile([P, NT * GW], f32, name="dbgg")
                    nc.vector.tensor_copy(dbg4[:], G_all[:])
                    nc.sync.dma_start(DBG_G[:, :], dbg4[:])
                p_t = stream.tile([P, N], fp16, tag="p", bufs=KNOBS["p_bufs"],
                                  name=f"p_{h}_{k}")
                eng = nc.gpsimd if use_pool else nc.vector
                eng.tensor_tensor(
                    p_t[:], u_t[:], AT_sb[:, k * N : (k + 1) * N], op.mult
                )
                return p_t

            def emit_aggs(h, st, k, p_t):
                G_all = st["G_all"]
                if st["agg"] is None:
                    st["agg"] = psu.tile([FH + 1, N], f32, tag="ps",
                                         name=f"agg{h}")
                agg = st["agg"]
                for c in range(NCH):
                    sl = slice(c * C, (c + 1) * C)
                    nc.tensor.matmul(
                        agg[:, sl],
                        G_all[:, k * GW : k * GW + FH + 1],
                        p_t[:, sl],
                        start=(k == 0), stop=(k == NT - 1),
                    )


            def emit_finals(h, st):
                agg = st["agg"]
                rrow = head.tile([1, N], f32, tag="rrow", bufs=1,
                                 name=f"rrow_{h}")
                if KNOBS["finals"] == "recip":
                    # custom-DVE recip misreads PSUM rows at partition
                    # offset 64 -> bounce den through SBUF partition 0
                    den_sb = head.tile([1, N], f32, tag="den_sb", bufs=1,
                                       name=f"den_sb_{h}")
                    nc.scalar.copy(den_sb[:], agg[FH : FH + 1, :])
                    if KNOBS.get("debug"):
                        nc.sync.dma_start(DBG_den[h : h + 1, :], den_sb[:])
                    nc.vector.reciprocal_approx_fast(rrow[:], den_sb[:])
                else:
                    lnr = head.tile([1, N], f32, tag="lnr", bufs=2,
                                    name=f"lnr_{h}")
                    nc.scalar.activation(lnr[:], agg[FH : FH + 1, :], act.Ln)
                    nc.scalar.activation(rrow[:], lnr[:], act.Exp, scale=-1.0)
                if KNOBS.get("debug"):
                    nc.sync.dma_start(DBG_rrow[h : h + 1, :], rrow[:])
                rbc = head.tile([FH, N], f32, tag="rbc", bufs=1,
                                name=f"rbc_{h}")
                nc.gpsimd.partition_broadcast(rbc[:], rrow[:])
                for c in range(NCH):
                    sl = slice(c * C, (c + 1) * C)
                    outf = outp.tile([FH, C], out_dt, tag="outf",
                                     name=f"outf_{h}_{c}")
                    nc.vector.scalar_tensor_tensor(
                        outf[:],
                        agg[0:FH, sl],
                        0.0, rbc[:, sl], op.max, op.mult,
                    )
                    nc.sync.dma_start(OUT_d[h, :, sl], outf[:])

            # ---- schedule ------------------------------------------------
            # all four setups run before any agg PSUM slot is claimed; heads
            # 0/1 aggregate while A^T streams in, heads 2/3 afterwards.
            # Pool-assigned p ops are emitted with lookahead so the in-order
            # PE agg queue never waits on the slower Pool engine.
            sts = [emit_setup(h) for h in range(H)]
            seq1 = [(h, k) for k in range(NT) for h in (0, 1)]
            seq2 = [(h, k) for k in range(NT) for h in (2, 3)]

            def pool_set(seq, stride):
                if not stride:
                    return set()
                return {hk for i, hk in enumerate(seq) if i % stride == stride - 1}

            def run_phase(seq, pool_tiles, la, per_k=None):
                pend = {}
                emitted = set()

                def produce(idx):
                    if idx >= len(seq) or seq[idx] in emitted:
                        return
                    h, k = seq[idx]
                    if seq[idx] in pool_tiles:
                        emitted.add(seq[idx])
                        pend[(h, k)] = emit_up(h, sts[h], k, True)

                lastk = -1
                for i, (h, k) in enumerate(seq):
                    if per_k is not None and k != lastk:
                        per_k(k)
                        lastk = k
                    for j in range(i, min(i + la + 1, len(seq))):
                        produce(j)
                    if (h, k) in pend:
                        emit_aggs(h, sts[h], k, pend.pop((h, k)))
                    else:
                        emit_aggs(h, sts[h], k, emit_up(h, sts[h], k, False))

            lead = KNOBS["lead"]
            merged = [0]

            def per_k1(k):
                while merged[0] < min(k + lead + 1, NT):
                    emit_merge_tile(merged[0])
                    merged[0] += 1

            pool1 = pool_set(seq1, KNOBS["p_pool_1"])
            pool2 = pool_set(seq2, KNOBS["p_pool_2"])
            run_phase(seq1, pool1, KNOBS["pool_la"], per_k=per_k1)
            emit_finals(0, sts[0])
            emit_finals(1, sts[1])
            run_phase(seq2, pool2, KNOBS["pool_la"])
            emit_finals(2, sts[2])
            emit_finals(3, sts[3])

    nc.compile()
    return nc


def _get_nc():
    if "nc" not in _CACHE:
        _CACHE["nc"] = _build()
    return _CACHE["nc"]


def make_in_maps(inputs):
    X = np.ascontiguousarray(inputs["X"], dtype=np.float32)
    A = np.ascontiguousarray(inputs["A"], dtype=np.float32)
    W = np.ascontiguousarray(inputs["W"], dtype=np.float32)
    b = np.ascontiguousarray(inputs["b"], dtype=np.float32)
    a_self = np.ascontiguousarray(inputs["a_self"], dtype=np.float32)
    a_neigh = np.ascontiguousarray(inputs["a_neigh"], dtype=np.float32)
    return [
        {
            "A": np.ascontiguousarray(A[i]),
            "X": np.ascontiguousarray(X[i]),
            "W": W,
            "b": b,
            "a_self": a_self,
            "a_neigh": a_neigh,
        }
        for i in range(B)
    ]


def run(inputs, trace=False):
    from concourse import bass_utils

    nc = _get_nc()
    in_maps = make_in_maps(inputs)
    res = bass_utils.run_bass_kernel_spmd(
        nc, in_maps, core_ids=list(range(B)), trace=trace
    )
    out = np.empty((B, N, H * FH), dtype=np.float32)
    for i in range(B):
        o = np.asarray(res.results[i]["OUT"], dtype=np.float32)  # [H, FH, N]
        out[i] = o.transpose(2, 0, 1).reshape(N, H * FH)
    return out, res


def kernel(**inputs):
    out, _ = run(inputs, trace=False)
    return out
